# revision 25
# baseline (speedup 1.0000x reference)
"""Trainium2 Bass kernel for a 16-head self-attention block.

Model (matches the nn.Module reference):
    q = x @ Wq + bq; k = x @ Wk + bk; v = x @ Wv + bv   (per-head split, Hd=64)
    attn = softmax(q k^T / sqrt(Hd)); out = (attn v) @ Wo + bo
Shapes: x [2, 2048, 1024], 16 heads, head dim 64.

Sharding (8 cores): core = (batch b in {0,1}) x (head-group g in {0..3});
each core owns 4 heads of one batch element. Inputs are sliced on the host;
each core returns a partial y^T = (attended_g @ Wo_g)^T which the host sums
over the 4 head-groups per batch.

Per-core design (all PE operands bf16; PSUM/normalization fp32):
  - Host passes x^T pre-chunked SEQ-MAJOR [128, 4 seq-blocks, 8 d-chunks,
    512] so the first Q/K projection slabs (and the whole qb0 stream) gate
    on just the first 1MB x block, not the whole 4MB tensor. Every
    projection consumer needs (all d-chunks x one seq block), so per-block
    DMAs unlock compute incrementally with 8KB-per-partition descriptors.
  - Input DMAs are ONE dma_start per tensor/block: the hardware sprays a
    [128, ...] transfer across all 16 DMA engines on its own, and each
    dma_start costs ~0.7us of issue time on its queue, so many small DMAs
    serialize on the issuing engine (the old 33-issue prologue spent ~25us
    just issuing). Weight DMAs issue from GpSimd, x blocks from Sync.
  - Scores are computed transposed, S^T[key, q] = K_h Q_h^T, so softmax's
    exp runs straight out of PSUM on the Scalar engine and A = P V consumes
    P^T with no transpose anywhere. Two heads of a pair share each score
    matmul slab via PE row groups (K=64 at row offsets 0/64).
  - softmax skips the max subtraction (mathematically identical; scores are
    O(5) here and ACT exp is <=2 ULP on [-10,10]).
  - P row sums ride the A = P V matmul via a ones column in V ([V|1] ->
    rows 0..63 attended + row 64 sums).
  - The exp (Scalar/ACT) stream is the pacing engine in steady state
    (~1.4us per key chunk, 128 chunks). To keep it stall-free the attended
    accumulators are RELEASED FAST: right after the AV stop-chunk, DVE
    casts av_ps[h][0:64] to SBUF (bf16) and copies the sums row to
    partition 0 (fp32); the PSUM banks free ~1.4us after the last AV
    instead of ~4us after the full normalization chain, so the next pair's
    AV (which reuses the same 2 banks) never back-pressures the exp stream
    through the pt WAR. pt pool is 6 deep for the same reason.
  - Normalization off PSUM: reciprocal_approx_fast on DVE, partition-
    broadcast on the (otherwise idle) GpSimd engine, one DVE multiply per
    head (bf16 attended x fp32 broadcast -> bf16 at_pair).
  - 1/sqrt(Hd) is folded into Wq (and bq) on the host; bv and bo are folded
    in exactly on the host: y += bo + bv @ Wo (softmax rows sum to 1).
  - Output projection of block qb is emitted a few chunks into block qb+1
    at low priority so its matmuls fill PE slack. For the LAST block the
    evictions alternate Scalar/Vector (both idle by then) to shorten the
    tail, and each y block goes out as one whole-tensor spray DMA.
  - A short dummy-matmul chain at t=0 ramps the PE clock while input DMAs
    stream.
"""

import numpy as np
import ml_dtypes

import concourse.bass as bass
import concourse.tile as tile
from concourse import bacc
from concourse import mybir

P = 128          # partitions
S = 2048         # sequence length
D = 1024         # model dim
H = 16           # total heads
HD = 64          # head dim
G = 4            # heads per core
GD = G * HD      # 256 head-group dims per core
NQB = 4          # query blocks (= seq blocks)
QB = S // NQB    # 512
NKC = S // P     # 16 key chunks
NDC = D // P     # 8 contraction chunks
F32 = mybir.dt.float32
BF16 = mybir.dt.bfloat16
DT = BF16        # PE operand dtype
NPDT = ml_dtypes.bfloat16
N_WARM = 12      # PE clock-ramp dummy matmuls (covers the ~13us DMA prologue)

TRACE = False
LAST_RESULTS = None


def _build_nc():
    nc = bacc.Bacc(trn_type="TRN2")
    xd = nc.dram_tensor("xd", [P, NQB, NDC, QB], DT, kind="ExternalInput")
    wq = nc.dram_tensor("wq", [P, NDC, GD], DT, kind="ExternalInput")
    wk = nc.dram_tensor("wk", [P, NDC, GD], DT, kind="ExternalInput")
    wv = nc.dram_tensor("wv", [P, NDC, GD], DT, kind="ExternalInput")
    wo = nc.dram_tensor("wo", [P, 2, D], DT, kind="ExternalInput")
    bias = nc.dram_tensor("bias", [P, 4], F32, kind="ExternalInput")
    yo = nc.dram_tensor("yo", [P, NQB, NDC, QB], DT, kind="ExternalOutput")

    Exp = mybir.ActivationFunctionType.Exp

    with tile.TileContext(nc) as tc, \
         tc.tile_pool(name="sb", bufs=1) as sb, \
         tc.tile_pool(name="pt", bufs=7) as ptp, \
         tc.tile_pool(name="attnp", bufs=5) as atp, \
         tc.tile_pool(name="avsbp", bufs=4) as avs, \
         tc.tile_pool(name="normp", bufs=4) as nrm, \
         tc.tile_pool(name="ysbp", bufs=2) as ysp, \
         tc.tile_pool(name="ps_s", bufs=2, space="PSUM") as ps_s, \
         tc.tile_pool(name="ps_av", bufs=2, space="PSUM") as ps_av, \
         tc.tile_pool(name="ps_y", bufs=2, space="PSUM") as ps_y:

        # ---- persistent SBUF tensors
        wq_sb = sb.tile([P, NDC, GD], DT, tag="wq")
        wk_sb = sb.tile([P, NDC, GD], DT, tag="wk")
        wv_sb = sb.tile([P, NDC, GD], DT, tag="wv")
        wo_sb = sb.tile([P, 2, D], DT, tag="wo")   # [pair-dims, pair, out-dim]
        bias_sb = sb.tile([P, 4], F32, tag="bias")
        scratch = sb.tile([P, 1], F32, tag="scratch")
        warm = sb.tile([P, QB], DT, tag="warm")
        x_sb = sb.tile([P, NQB, NDC, QB], DT, tag="x")
        kT = [sb.tile([P, S], DT, tag=f"k{p}", name=f"k{p}") for p in range(2)]
        qT = [sb.tile([P, S], DT, tag=f"q{p}", name=f"q{p}") for p in range(2)]
        # V with a ones column per head: [keys, chunk, head, 65] = [V | 1]
        v_sb = sb.tile([P, NKC, G, HD + 1], DT, tag="v")

        # ---- warm tiles + PE ramp (no input deps: runs during the DMA
        # prologue so the PE clock is at speed when projections start)
        nc.vector.memset(warm, 0.0)
        nc.vector.memset(v_sb[:, :, :, HD:HD + 1], 1.0)
        # warm the exp table set early so the ~2.7us load overlaps the DMAs
        nc.scalar.activation(out=scratch, in_=warm[:, 0:1], func=Exp)
        with tc.high_priority(offset=-1000000):
            for i in range(N_WARM):
                wps = ps_y.tile([P, QB], F32, tag="y", name="warm_ps")
                nc.tensor.matmul(wps[:], lhsT=warm[:, 0:P], rhs=warm[:],
                                 start=True, stop=True)

        # ---- input DMAs. One spray-DMA per tensor / x block (the DMA
        # hardware sprays each [128, ...] transfer across all 16 engines);
        # per-queue FIFO makes issue order the arrival order, so both queues
        # are laid out in consumption order: weights on GpSimd (wq before
        # wk — the first projection slab is Q), x blocks on Sync. The two
        # queues share HBM bandwidth ~evenly, landing wq ~8us in and x
        # block b at ~10+5b us — each just ahead of its first consumer.
        # (Keep the two-queue split: routing everything through one queue
        # measurably slowed every ACTIVATE by ~220ns and serialized the
        # Sync engine. And use the Scalar HW queue for weights, NOT the
        # GpSimd software queue — the latter moves only ~55 GB/s.)
        # wq/wk must beat x block 0 (they gate the very first projections)
        # so they lead the fast Sync queue; wv/wo ride the slower Scalar
        # queue concurrently and still land well before their consumers.
        nc.sync.dma_start(out=wq_sb, in_=wq[:, :, :])
        nc.sync.dma_start(out=wk_sb, in_=wk[:, :, :])
        nc.sync.dma_start(out=bias_sb, in_=bias[:, :])
        nc.scalar.dma_start(out=wv_sb, in_=wv[:, :, :])
        nc.scalar.dma_start(out=wo_sb, in_=wo[:, :, :])
        for b in range(NQB):
            nc.sync.dma_start(out=x_sb[:, b], in_=xd[:, b])

        # (No weight-DMA pre-observation matmuls needed: each tensor is ONE
        # DMA with ONE completion semaphore, and every matmul pair spreads
        # its two input waits across LDWEIGHTS + MATMUL.)

        # ---- projection emitters
        def emit_qk_group(w_sb, dst, bcol0, p, blk):
            # one [128, 512] output slab of K^T or Q^T; dst[p] [128, 2048]
            # rows 64*h2 hold head (2p+h2)'s 64 dims, columns are sequence.
            # Depends on x seq-block blk only. Allocated from the ps_y pool
            # so the score pool is never blocked behind projection
            # evictions.
            n0 = blk * QB
            ps = ps_y.tile([P, QB], F32, tag="y", name="qk_ps")
            for d in range(NDC):
                nc.tensor.matmul(
                    ps[:],
                    lhsT=w_sb[:, d, p * P:(p + 1) * P],
                    rhs=x_sb[:, blk, d, :],
                    start=(d == 0), stop=(d == NDC - 1))
            # evict with per-partition bias add on the DVE (keeps the Scalar
            # engine free to run the exp stream from its very first chunk)
            with nc.allow_low_precision(reason="bf16 projection"):
                nc.vector.tensor_scalar_add(
                    out=dst[p][:, n0:n0 + QB],
                    in0=ps[:],
                    scalar1=bias_sb[:, bcol0 + p:bcol0 + p + 1])

        def emit_v_chunk(c):
            blk, c0 = c // 4, (c % 4) * P
            ps = ps_y.tile([P, GD], F32, tag="y", name="v_ps")
            for d in range(NDC):
                nc.tensor.matmul(
                    ps[:],
                    lhsT=x_sb[:, blk, d, c0:c0 + P],
                    rhs=wv_sb[:, d, :],
                    start=(d == 0), stop=(d == NDC - 1))
            nc.vector.tensor_copy(
                out=v_sb[:, c, :, 0:HD],
                in_=ps[:].rearrange("p (h d) -> p h d", h=G))

        # Engines execute their static streams IN ORDER, so every
        # projection group must be emitted at the point its x block lands —
        # never earlier (it would block the stream behind its DMA wait) and
        # never later than its first consumer. Only pair-0's slab-0 K and Q
        # groups and V chunk 0 precede the attention loop: the first exp —
        # which starts the Scalar stream that paces the whole kernel —
        # fires as soon as x block 0 is in. V chunks 1..15 and the later K
        # slabs interleave into the qb0 chunk loops right where needed.
        emit_qk_group(wq_sb, qT, 0, 0, 0)
        emit_qk_group(wk_sb, kT, 2, 0, 0)
        emit_v_chunk(0)

        # ---- attention + output projection: per query block, head pairs
        # processed sequentially (pass p covers heads 2p, 2p+1). The output
        # projection of block qb is emitted a few chunks into block qb+1 so
        # its matmuls fill PE slack instead of stalling the exp stream.
        pending_outproj = None
        for qb in range(NQB):
            q0 = qb * QB
            attn = []
            for p in range(2):
                av_ps = [ps_av.tile([P, QB], F32, tag="av", name="av_ps")
                         for _ in range(2)]
                for c in range(NKC):
                    if qb == 0 and p == 0:
                        # ALL qb0 projection work lives in pair 0's loop
                        # (which is PE-oversubscribed anyway): pair 1's K/Q
                        # slabs here keep pair 1's stream pure attention so
                        # the p0->p1 handoff costs the exp stream nothing.
                        if c in (4, 8, 12):
                            emit_qk_group(wk_sb, kT, 2, 0, c // 4)
                        if c in (5, 9, 13):
                            emit_qk_group(wk_sb, kT, 2, 1, c // 4)
                        if c >= 1:
                            emit_v_chunk(c)  # V just ahead of its first AV
                        if c == 1:
                            emit_qk_group(wk_sb, kT, 2, 1, 0)
                        if c == 3:
                            emit_qk_group(wq_sb, qT, 0, 1, 0)
                        if c == 2:
                            # qb1 queries; x block 1 nearly in — slack filler
                            with tc.high_priority(offset=-1000000):
                                emit_qk_group(wq_sb, qT, 0, 0, 1)
                        if c == 6:
                            with tc.high_priority(offset=-1000000):
                                emit_qk_group(wq_sb, qT, 0, 1, 1)
                    if pending_outproj is not None and p == 0 and c == 3:
                        pending_outproj()
                        pending_outproj = None
                    c0 = c * P
                    s_ps = ps_s.tile([P, 2, QB], F32, tag="s")
                    for h2 in range(2):
                        base = HD * h2
                        nc.tensor.matmul(
                            s_ps[:, h2],
                            lhsT=kT[p][base:base + HD, c0:c0 + P],
                            rhs=qT[p][base:base + HD, q0:q0 + QB],
                            start=True, stop=True,
                            tile_position=(base, 0))
                    pt = ptp.tile([P, 2, QB], DT, tag="pt")
                    nc.scalar.activation(out=pt[:], in_=s_ps[:], func=Exp)
                    for h2 in range(2):
                        nc.tensor.matmul(
                            av_ps[h2][0:HD + 1, :],
                            lhsT=v_sb[:, c, 2 * p + h2, :],
                            rhs=pt[:, h2],
                            start=(c == 0), stop=(c == NKC - 1))

                # Normalization. The av tile packs both heads side by side
                # (banks h=0/1), so one DVE op covers both heads for the
                # attended-rows cast to SBUF bf16 (FIRST, so the 2 av banks
                # release ~2.7us after the last AV matmul — the next pair's
                # AV start rides on that while the 7-deep pt pool keeps the
                # exp stream ahead) and for the sums-row copy to partition 0
                # (fp32; a plain DVE copy can partition-base shift, the
                # custom recip op cannot). Reciprocal on DVE, partition
                # broadcast on the otherwise-idle GpSimd, one DVE multiply
                # per head. For the LAST pair there is no next-pair release
                # pressure, so the cast is skipped and the multiplies read
                # the attended rows straight from PSUM.
                last = qb == NQB - 1 and p == 1
                if last:
                    # keep the PE clock hot through the final normalization
                    # chain (~5us of otherwise-idle PE would re-throttle the
                    # HAM and run the last output projection at half clock).
                    # Reading the last pt tile pins these after the final
                    # exp — dependency-free fillers get list-scheduled into
                    # earlier slack and miss this window entirely.
                    for i in range(10):
                        wps = ps_y.tile([P, QB], F32, tag="y",
                                        name="tail_warm_ps")
                        nc.tensor.matmul(wps[:], lhsT=warm[:, 0:P],
                                         rhs=pt[:, 0], start=True, stop=True)
                av_sb = [avs.tile([HD, QB], DT, tag=f"avsb{h}",
                                  name=f"avsb{h}") for h in range(2)]
                rr = [nrm.tile([1, QB], F32, tag=f"rr{h}", name=f"rr{h}")
                      for h in range(2)]
                rc = [nrm.tile([1, QB], F32, tag=f"rc{h}", name=f"rc{h}")
                      for h in range(2)]
                bc = [nrm.tile([HD, QB], F32, tag=f"bc{h}", name=f"bc{h}")
                      for h in range(2)]
                at_pair = atp.tile([P, QB], DT, tag="attn")
                with nc.allow_low_precision(reason="softmax denom approx"):
                    for h in range(2):
                        if not last:
                            nc.vector.tensor_copy(out=av_sb[h][:],
                                                  in_=av_ps[h][0:HD, :])
                        nc.vector.tensor_copy(out=rr[h][:],
                                              in_=av_ps[h][HD:HD + 1, :])
                        nc.vector.reciprocal_approx_fast(out=rc[h][:],
                                                         in_=rr[h][:])
                        nc.gpsimd.partition_broadcast(bc[h][:, :], rc[h][:, :])
                    for h in range(2):
                        # partition-base shift 0 -> 64 on the DVE packs the
                        # odd head into the pair tile with no relocation DMA
                        nc.vector.tensor_tensor(
                            out=at_pair[h * HD:(h + 1) * HD, :],
                            in0=(av_ps[h][0:HD, :] if last
                                 else av_sb[h][:]),
                            in1=bc[h][:, :],
                            op=mybir.AluOpType.mult)
                attn.append(at_pair)
                # qb2/qb3 queries, deprioritized so they only fill PE slack
                # in the later (projection-free, ACT-paced) query blocks
                if qb in (1, 2):
                    with tc.high_priority(offset=-1000000):
                        emit_qk_group(wq_sb, qT, 0, p, qb + 1)

            def emit_outproj(attn=attn, qb=qb):
                # y^T[m-chunk, qb] = sum_p Wo_p^T @ attn_pair_p.
                # Deprioritized: these matmuls fill PE slack so they never
                # delay the score matmuls that feed the exp stream. For the
                # last block each eviction is split across Scalar+Vector
                # (both idle once the exp stream ends) so the yp PSUM pair
                # recycles ~2x faster, and the y DMA goes out in two halves
                # to overlap the final evictions.
                last = qb == NQB - 1
                ctx2 = tc.high_priority(offset=-1000000)
                ctx2.__enter__()
                ysb = ysp.tile([P, NDC, QB], DT, tag="ysb")
                for m in range(NDC):
                    yp = ps_y.tile([P, QB], F32, tag="y", name="yp")
                    for h in range(2):
                        nc.tensor.matmul(
                            yp[:],
                            lhsT=wo_sb[:, h, m * P:(m + 1) * P],
                            rhs=attn[h][:],
                            start=(h == 0), stop=(h == 1))
                    with nc.allow_low_precision(reason="bf16 partial out"):
                        if last:
                            nc.scalar.copy(out=ysb[:, m, 0:QB // 2],
                                           in_=yp[:, 0:QB // 2])
                            nc.vector.tensor_copy(out=ysb[:, m, QB // 2:],
                                                  in_=yp[:, QB // 2:])
                        else:
                            nc.vector.tensor_copy(out=ysb[:, m, :], in_=yp[:])
                    if last and m == NDC // 2 - 1:
                        nc.sync.dma_start(out=yo[:, qb, 0:NDC // 2, :],
                                          in_=ysb[:, 0:NDC // 2, :])
                # whole-tensor spray DMAs (8KB per-partition descriptors)
                if last:
                    nc.sync.dma_start(out=yo[:, qb, NDC // 2:, :],
                                      in_=ysb[:, NDC // 2:, :])
                else:
                    nc.sync.dma_start(out=yo[:, qb, :, :], in_=ysb[:, :, :])
                ctx2.__exit__(None, None, None)

            pending_outproj = emit_outproj

        if pending_outproj is not None:
            pending_outproj()

    nc.compile()
    return nc


_CACHE = {}


def _get_nc():
    if "nc" not in _CACHE:
        _CACHE["nc"] = _build_nc()
    return _CACHE["nc"]


def make_in_maps(x, Wq, bq, Wk, bk, Wv, bv, Wo, bo):
    """Host-side sharding: per-core input dicts for cores 0..7."""
    x = np.asarray(x, np.float32)
    scale = np.float32(1.0 / np.sqrt(HD))
    Wq_s = np.asarray(Wq, np.float32) * scale
    bq_s = np.asarray(bq, np.float32) * scale
    Wk = np.asarray(Wk, np.float32)
    bk = np.asarray(bk, np.float32)
    Wv = np.asarray(Wv, np.float32)
    Wo = np.asarray(Wo, np.float32)

    def chunk_rows(w):  # [1024, M] -> [128, 8, M]
        return np.ascontiguousarray(
            w.reshape(NDC, P, w.shape[1]).transpose(1, 0, 2)).astype(NPDT)

    # x^T seq-major: [128 d-in-chunk, 4 seq-block, 8 d-chunk, 512 seq]
    xds = [np.ascontiguousarray(
               chunk_rows(x[b].T).reshape(P, NDC, NQB, QB).transpose(0, 2, 1, 3))
           for b in range(2)]
    in_maps = []
    for core in range(8):
        b, g = divmod(core, 4)
        cols = slice(g * GD, (g + 1) * GD)
        bias = np.zeros((P, 4), np.float32)
        bias[:, 0] = bq_s[g * GD:g * GD + P]
        bias[:, 1] = bq_s[g * GD + P:(g + 1) * GD]
        bias[:, 2] = bk[g * GD:g * GD + P]
        bias[:, 3] = bk[g * GD + P:(g + 1) * GD]
        in_maps.append({
            "xd": xds[b],
            "wq": chunk_rows(Wq_s[:, cols]),
            "wk": chunk_rows(Wk[:, cols]),
            "wv": chunk_rows(Wv[:, cols]),
            "wo": np.ascontiguousarray(
                Wo[cols, :].reshape(2, P, D).transpose(1, 0, 2)).astype(NPDT),
            "bias": bias,
        })
    return in_maps


def gather_output(results, Wv, bv, Wo, bo):
    """Sum per-core partial outputs and fold bv/bo exactly."""
    y = np.zeros((2, S, D), np.float32)
    for core in range(8):
        b = core // 4
        # yo [128 p, 4 qb, 8 m, 512 col] -> [qb*512+col, m*128+p] = [s, d]
        yo = np.asarray(results[core]["yo"], dtype=np.float32)
        y[b] += yo.transpose(1, 3, 2, 0).reshape(S, D)
    y += np.asarray(bo, np.float32) + np.asarray(bv, np.float32) @ np.asarray(Wo, np.float32)
    return y


def kernel(x, Wq, bq, Wk, bk, Wv, bv, Wo, bo):
    global LAST_RESULTS
    from concourse.bass_utils import run_bass_kernel_spmd
    in_maps = make_in_maps(x, Wq, bq, Wk, bk, Wv, bv, Wo, bo)
    res = run_bass_kernel_spmd(_get_nc(), in_maps, core_ids=list(range(8)),
                               trace=TRACE)
    LAST_RESULTS = res
    return gather_output(res.results, Wv, bv, Wo, bo)


# revision 28
# speedup vs baseline: 1.1961x; 1.1961x over previous
"""Trainium2 Bass kernel for a 16-head self-attention block.

Model (matches the nn.Module reference):
    q = x @ Wq + bq; k = x @ Wk + bk; v = x @ Wv + bv   (per-head split, Hd=64)
    attn = softmax(q k^T / sqrt(Hd)); out = (attn v) @ Wo + bo
Shapes: x [2, 2048, 1024], 16 heads, head dim 64.

Sharding (8 cores): core = (batch b in {0,1}) x (head-group g in {0..3});
each core owns 4 heads of one batch element. Inputs are sliced on the host;
each core returns a partial y^T = (attended_g @ Wo_g)^T which the host sums
over the 4 head-groups per batch.

Per-core design (all PE operands bf16; PSUM/normalization fp32):
  - Host passes x^T pre-chunked SEQ-MAJOR [128, 4 seq-blocks, 8 d-chunks,
    512] so the first Q/K projection slabs (and the whole qb0 stream) gate
    on just the first 1MB x block, not the whole 4MB tensor. Every
    projection consumer needs (all d-chunks x one seq block), so per-block
    DMAs unlock compute incrementally with 8KB-per-partition descriptors.
  - Input DMAs are ONE dma_start per tensor/block: the hardware sprays a
    [128, ...] transfer across all 16 DMA engines on its own, and each
    dma_start costs ~0.7us of issue time on its queue, so many small DMAs
    serialize on the issuing engine (the old 33-issue prologue spent ~25us
    just issuing). Weight DMAs issue from GpSimd, x blocks from Sync.
  - Scores are computed transposed, S^T[key, q] = K_h Q_h^T, so softmax's
    exp runs straight out of PSUM on the Scalar engine and A = P V consumes
    P^T with no transpose anywhere. Two heads of a pair share each score
    matmul slab via PE row groups (K=64 at row offsets 0/64).
  - softmax skips the max subtraction (mathematically identical; scores are
    O(5) here and ACT exp is <=2 ULP on [-10,10]).
  - P row sums ride the A = P V matmul via a ones column in V ([V|1] ->
    rows 0..63 attended + row 64 sums).
  - The exp (Scalar/ACT) stream is the pacing engine in steady state
    (~1.4us per key chunk, 128 chunks). To keep it stall-free the attended
    accumulators are RELEASED FAST: right after the AV stop-chunk, DVE
    casts av_ps[h][0:64] to SBUF (bf16) and copies the sums row to
    partition 0 (fp32); the PSUM banks free ~1.4us after the last AV
    instead of ~4us after the full normalization chain, so the next pair's
    AV (which reuses the same 2 banks) never back-pressures the exp stream
    through the pt WAR. pt pool is 6 deep for the same reason.
  - Normalization off PSUM: reciprocal_approx_fast on DVE, partition-
    broadcast on the (otherwise idle) GpSimd engine, one DVE multiply per
    head (bf16 attended x fp32 broadcast -> bf16 at_pair).
  - 1/sqrt(Hd) is folded into Wq (and bq) on the host; bv and bo are folded
    in exactly on the host: y += bo + bv @ Wo (softmax rows sum to 1).
  - Output projection of block qb is emitted a few chunks into block qb+1
    at low priority so its matmuls fill PE slack. For the LAST block the
    evictions alternate Scalar/Vector (both idle by then) to shorten the
    tail, and each y block goes out as one whole-tensor spray DMA.
  - A short dummy-matmul chain at t=0 ramps the PE clock while input DMAs
    stream.
"""

import numpy as np
import ml_dtypes

import concourse.bass as bass
import concourse.tile as tile
from concourse import bacc
from concourse import mybir

P = 128          # partitions
S = 2048         # sequence length
D = 1024         # model dim
H = 16           # total heads
HD = 64          # head dim
G = 4            # heads per core
GD = G * HD      # 256 head-group dims per core
NQB = 4          # query blocks (= seq blocks)
QB = S // NQB    # 512
NKC = S // P     # 16 key chunks
NDC = D // P     # 8 contraction chunks
F32 = mybir.dt.float32
BF16 = mybir.dt.bfloat16
DT = BF16        # PE operand dtype
NPDT = ml_dtypes.bfloat16
N_WARM = 12      # PE clock-ramp dummy matmuls (covers the ~13us DMA prologue)

TRACE = False
LAST_RESULTS = None


def _build_nc():
    nc = bacc.Bacc(trn_type="TRN2")
    xd = nc.dram_tensor("xd", [P, NQB, NDC, QB], DT, kind="ExternalInput")
    wq = nc.dram_tensor("wq", [P, NDC, GD], DT, kind="ExternalInput")
    wk = nc.dram_tensor("wk", [P, NDC, GD], DT, kind="ExternalInput")
    wv = nc.dram_tensor("wv", [P, NDC, GD], DT, kind="ExternalInput")
    wo = nc.dram_tensor("wo", [P, 2, D], DT, kind="ExternalInput")
    bias = nc.dram_tensor("bias", [P, 4], F32, kind="ExternalInput")
    yo = nc.dram_tensor("yo", [P, NQB, NDC, QB], DT, kind="ExternalOutput")

    Exp = mybir.ActivationFunctionType.Exp

    with tile.TileContext(nc) as tc, \
         tc.tile_pool(name="sb", bufs=1) as sb, \
         tc.tile_pool(name="pt", bufs=7) as ptp, \
         tc.tile_pool(name="attnp", bufs=5) as atp, \
         tc.tile_pool(name="avsbp", bufs=4) as avs, \
         tc.tile_pool(name="normp", bufs=4) as nrm, \
         tc.tile_pool(name="ysbp", bufs=2) as ysp, \
         tc.tile_pool(name="ps_s", bufs=2, space="PSUM") as ps_s, \
         tc.tile_pool(name="ps_av", bufs=2, space="PSUM") as ps_av, \
         tc.tile_pool(name="ps_y", bufs=2, space="PSUM") as ps_y:

        # ---- persistent SBUF tensors
        wq_sb = sb.tile([P, NDC, GD], DT, tag="wq")
        wk_sb = sb.tile([P, NDC, GD], DT, tag="wk")
        wv_sb = sb.tile([P, NDC, GD], DT, tag="wv")
        wo_sb = sb.tile([P, 2, D], DT, tag="wo")   # [pair-dims, pair, out-dim]
        bias_sb = sb.tile([P, 4], F32, tag="bias")
        scratch = sb.tile([P, 1], F32, tag="scratch")
        warm = sb.tile([P, QB], DT, tag="warm")
        x_sb = sb.tile([P, NQB, NDC, QB], DT, tag="x")
        kT = [sb.tile([P, S], DT, tag=f"k{p}", name=f"k{p}") for p in range(2)]
        qT = [sb.tile([P, S], DT, tag=f"q{p}", name=f"q{p}") for p in range(2)]
        # V with a ones column per head: [keys, chunk, head, 65] = [V | 1]
        v_sb = sb.tile([P, NKC, G, HD + 1], DT, tag="v")

        # ---- warm tiles + PE ramp (no input deps: runs during the DMA
        # prologue so the PE clock is at speed when projections start)
        nc.vector.memset(warm, 0.0)
        nc.vector.memset(v_sb[:, :, :, HD:HD + 1], 1.0)
        # warm the exp table set early so the ~2.7us load overlaps the DMAs
        nc.scalar.activation(out=scratch, in_=warm[:, 0:1], func=Exp)
        with tc.high_priority(offset=-1000000):
            for i in range(N_WARM):
                wps = ps_y.tile([P, QB], F32, tag="y", name="warm_ps")
                nc.tensor.matmul(wps[:], lhsT=warm[:, 0:P], rhs=warm[:],
                                 start=True, stop=True)

        # ---- input DMAs. One spray-DMA per tensor / x block (the DMA
        # hardware sprays each [128, ...] transfer across all 16 engines);
        # per-queue FIFO makes issue order the arrival order, so both queues
        # are laid out in consumption order: weights on GpSimd (wq before
        # wk — the first projection slab is Q), x blocks on Sync. The two
        # queues share HBM bandwidth ~evenly, landing wq ~8us in and x
        # block b at ~10+5b us — each just ahead of its first consumer.
        # (Keep the two-queue split: routing everything through one queue
        # measurably slowed every ACTIVATE by ~220ns and serialized the
        # Sync engine. And use the Scalar HW queue for weights, NOT the
        # GpSimd software queue — the latter moves only ~55 GB/s.)
        # wq/wk must beat x block 0 (they gate the very first projections)
        # so they lead the fast Sync queue; wv/wo ride the slower Scalar
        # queue concurrently and still land well before their consumers.
        nc.sync.dma_start(out=wq_sb, in_=wq[:, :, :])
        nc.sync.dma_start(out=wk_sb, in_=wk[:, :, :])
        nc.sync.dma_start(out=bias_sb, in_=bias[:, :])
        nc.scalar.dma_start(out=wv_sb, in_=wv[:, :, :])
        nc.scalar.dma_start(out=wo_sb, in_=wo[:, :, :])
        for b in range(NQB):
            nc.sync.dma_start(out=x_sb[:, b], in_=xd[:, b])

        # Pre-observe each weight DMA on the PE with a 1x1 dummy matmul, so
        # real matmuls never need two DMA-queue waits at once (walrus can't
        # encode >1 sync wait on an LDWEIGHTS; dropping these measurably
        # slowed every ACTIVATE by ~220ns). wq/wk are observed here — they
        # land first on the fast queue — while the slow-queue wv/wo
        # observations are emitted right before their first consumers so
        # they never stall the projection stream.
        wtouch_ps = ps_y.tile([1, 4], F32, tag="y", name="wtouch")

        def wtouch(w, i):
            nc.tensor.matmul(wtouch_ps[:, i:i + 1],
                             lhsT=w[0:1, 0, 0:1],
                             rhs=w[0:1, 0, 0:1],
                             start=True, stop=True)

        wtouch(wq_sb, 0)
        wtouch(wk_sb, 1)

        # ---- projection emitters
        def emit_qk_group(w_sb, dst, bcol0, p, blk):
            # one [128, 512] output slab of K^T or Q^T; dst[p] [128, 2048]
            # rows 64*h2 hold head (2p+h2)'s 64 dims, columns are sequence.
            # Depends on x seq-block blk only. Allocated from the ps_y pool
            # so the score pool is never blocked behind projection
            # evictions.
            n0 = blk * QB
            ps = ps_y.tile([P, QB], F32, tag="y", name="qk_ps")
            for d in range(NDC):
                nc.tensor.matmul(
                    ps[:],
                    lhsT=w_sb[:, d, p * P:(p + 1) * P],
                    rhs=x_sb[:, blk, d, :],
                    start=(d == 0), stop=(d == NDC - 1))
            # evict with per-partition bias add on the DVE (keeps the Scalar
            # engine free to run the exp stream from its very first chunk)
            with nc.allow_low_precision(reason="bf16 projection"):
                nc.vector.tensor_scalar_add(
                    out=dst[p][:, n0:n0 + QB],
                    in0=ps[:],
                    scalar1=bias_sb[:, bcol0 + p:bcol0 + p + 1])

        def emit_v_chunk(c):
            blk, c0 = c // 4, (c % 4) * P
            ps = ps_y.tile([P, GD], F32, tag="y", name="v_ps")
            for d in range(NDC):
                nc.tensor.matmul(
                    ps[:],
                    lhsT=x_sb[:, blk, d, c0:c0 + P],
                    rhs=wv_sb[:, d, :],
                    start=(d == 0), stop=(d == NDC - 1))
            nc.vector.tensor_copy(
                out=v_sb[:, c, :, 0:HD],
                in_=ps[:].rearrange("p (h d) -> p h d", h=G))

        # Engines execute their static streams IN ORDER, so every
        # projection group must be emitted at the point its x block lands —
        # never earlier (it would block the stream behind its DMA wait) and
        # never later than its first consumer. Only pair-0's slab-0 K and Q
        # groups and V chunk 0 precede the attention loop: the first exp —
        # which starts the Scalar stream that paces the whole kernel —
        # fires as soon as x block 0 is in. V chunks 1..15 and the later K
        # slabs interleave into the qb0 chunk loops right where needed.
        emit_qk_group(wq_sb, qT, 0, 0, 0)
        emit_qk_group(wk_sb, kT, 2, 0, 0)
        wtouch(wv_sb, 2)
        emit_v_chunk(0)

        # ---- attention + output projection: per query block, head pairs
        # processed sequentially (pass p covers heads 2p, 2p+1). The output
        # projection of block qb is emitted a few chunks into block qb+1 so
        # its matmuls fill PE slack instead of stalling the exp stream.
        pending_outproj = None
        for qb in range(NQB):
            q0 = qb * QB
            attn = []
            for p in range(2):
                av_ps = [ps_av.tile([P, QB], F32, tag="av", name="av_ps")
                         for _ in range(2)]
                for c in range(NKC):
                    if qb == 0 and p == 0:
                        # ALL qb0 projection work lives in pair 0's loop
                        # (which is PE-oversubscribed anyway): pair 1's K/Q
                        # slabs here keep pair 1's stream pure attention so
                        # the p0->p1 handoff costs the exp stream nothing.
                        if c in (4, 8, 12):
                            emit_qk_group(wk_sb, kT, 2, 0, c // 4)
                        if c in (5, 9, 13):
                            emit_qk_group(wk_sb, kT, 2, 1, c // 4)
                        if c >= 1:
                            emit_v_chunk(c)  # V just ahead of its first AV
                        if c == 1:
                            emit_qk_group(wk_sb, kT, 2, 1, 0)
                        if c == 3:
                            emit_qk_group(wq_sb, qT, 0, 1, 0)
                        if c == 2:
                            # qb1 queries; x block 1 nearly in — slack filler
                            with tc.high_priority(offset=-1000000):
                                emit_qk_group(wq_sb, qT, 0, 0, 1)
                        if c == 6:
                            with tc.high_priority(offset=-1000000):
                                emit_qk_group(wq_sb, qT, 0, 1, 1)
                        if c == 8:
                            wtouch(wo_sb, 3)
                    if pending_outproj is not None and p == 0 and c == 3:
                        pending_outproj()
                        pending_outproj = None
                    c0 = c * P
                    s_ps = ps_s.tile([P, 2, QB], F32, tag="s")
                    for h2 in range(2):
                        base = HD * h2
                        nc.tensor.matmul(
                            s_ps[:, h2],
                            lhsT=kT[p][base:base + HD, c0:c0 + P],
                            rhs=qT[p][base:base + HD, q0:q0 + QB],
                            start=True, stop=True,
                            tile_position=(base, 0))
                    pt = ptp.tile([P, 2, QB], DT, tag="pt")
                    nc.scalar.activation(out=pt[:], in_=s_ps[:], func=Exp)
                    for h2 in range(2):
                        nc.tensor.matmul(
                            av_ps[h2][0:HD + 1, :],
                            lhsT=v_sb[:, c, 2 * p + h2, :],
                            rhs=pt[:, h2],
                            start=(c == 0), stop=(c == NKC - 1))

                # Normalization. The av tile packs both heads side by side
                # (banks h=0/1), so one DVE op covers both heads for the
                # attended-rows cast to SBUF bf16 (FIRST, so the 2 av banks
                # release ~2.7us after the last AV matmul — the next pair's
                # AV start rides on that while the 7-deep pt pool keeps the
                # exp stream ahead) and for the sums-row copy to partition 0
                # (fp32; a plain DVE copy can partition-base shift, the
                # custom recip op cannot). Reciprocal on DVE, partition
                # broadcast on the otherwise-idle GpSimd, one DVE multiply
                # per head. For the LAST pair there is no next-pair release
                # pressure, so the cast is skipped and the multiplies read
                # the attended rows straight from PSUM.
                last = qb == NQB - 1 and p == 1
                if last:
                    # keep the PE clock hot through the final normalization
                    # chain (~5us of otherwise-idle PE would re-throttle the
                    # HAM and run the last output projection at half clock).
                    # Reading the last pt tile pins these after the final
                    # exp — dependency-free fillers get list-scheduled into
                    # earlier slack and miss this window entirely.
                    for i in range(10):
                        wps = ps_y.tile([P, QB], F32, tag="y",
                                        name="tail_warm_ps")
                        nc.tensor.matmul(wps[:], lhsT=warm[:, 0:P],
                                         rhs=pt[:, 0], start=True, stop=True)
                av_sb = [avs.tile([HD, QB], DT, tag=f"avsb{h}",
                                  name=f"avsb{h}") for h in range(2)]
                rr = [nrm.tile([1, QB], F32, tag=f"rr{h}", name=f"rr{h}")
                      for h in range(2)]
                rc = [nrm.tile([1, QB], F32, tag=f"rc{h}", name=f"rc{h}")
                      for h in range(2)]
                bc = [nrm.tile([HD, QB], F32, tag=f"bc{h}", name=f"bc{h}")
                      for h in range(2)]
                at_pair = atp.tile([P, QB], DT, tag="attn")
                with nc.allow_low_precision(reason="softmax denom approx"):
                    for h in range(2):
                        if not last:
                            nc.vector.tensor_copy(out=av_sb[h][:],
                                                  in_=av_ps[h][0:HD, :])
                        nc.vector.tensor_copy(out=rr[h][:],
                                              in_=av_ps[h][HD:HD + 1, :])
                        nc.vector.reciprocal_approx_fast(out=rc[h][:],
                                                         in_=rr[h][:])
                        nc.gpsimd.partition_broadcast(bc[h][:, :], rc[h][:, :])
                    for h in range(2):
                        # partition-base shift 0 -> 64 on the DVE packs the
                        # odd head into the pair tile with no relocation DMA
                        nc.vector.tensor_tensor(
                            out=at_pair[h * HD:(h + 1) * HD, :],
                            in0=(av_ps[h][0:HD, :] if last
                                 else av_sb[h][:]),
                            in1=bc[h][:, :],
                            op=mybir.AluOpType.mult)
                attn.append(at_pair)
                # qb2/qb3 queries, deprioritized so they only fill PE slack
                # in the later (projection-free, ACT-paced) query blocks
                if qb in (1, 2):
                    with tc.high_priority(offset=-1000000):
                        emit_qk_group(wq_sb, qT, 0, p, qb + 1)

            def emit_outproj(attn=attn, qb=qb):
                # y^T[m-chunk, qb] = sum_p Wo_p^T @ attn_pair_p.
                # Deprioritized: these matmuls fill PE slack so they never
                # delay the score matmuls that feed the exp stream. For the
                # last block each eviction is split across Scalar+Vector
                # (both idle once the exp stream ends) so the yp PSUM pair
                # recycles ~2x faster, and the y DMA goes out in two halves
                # to overlap the final evictions.
                last = qb == NQB - 1
                ctx2 = tc.high_priority(offset=-1000000)
                ctx2.__enter__()
                ysb = ysp.tile([P, NDC, QB], DT, tag="ysb")
                for m in range(NDC):
                    yp = ps_y.tile([P, QB], F32, tag="y", name="yp")
                    for h in range(2):
                        nc.tensor.matmul(
                            yp[:],
                            lhsT=wo_sb[:, h, m * P:(m + 1) * P],
                            rhs=attn[h][:],
                            start=(h == 0), stop=(h == 1))
                    with nc.allow_low_precision(reason="bf16 partial out"):
                        if last:
                            nc.scalar.copy(out=ysb[:, m, 0:QB // 2],
                                           in_=yp[:, 0:QB // 2])
                            nc.vector.tensor_copy(out=ysb[:, m, QB // 2:],
                                                  in_=yp[:, QB // 2:])
                        else:
                            nc.vector.tensor_copy(out=ysb[:, m, :], in_=yp[:])
                    if last and m == NDC // 2 - 1:
                        nc.sync.dma_start(out=yo[:, qb, 0:NDC // 2, :],
                                          in_=ysb[:, 0:NDC // 2, :])
                # whole-tensor spray DMAs (8KB per-partition descriptors)
                if last:
                    nc.sync.dma_start(out=yo[:, qb, NDC // 2:, :],
                                      in_=ysb[:, NDC // 2:, :])
                else:
                    nc.sync.dma_start(out=yo[:, qb, :, :], in_=ysb[:, :, :])
                ctx2.__exit__(None, None, None)

            pending_outproj = emit_outproj

        if pending_outproj is not None:
            pending_outproj()

    nc.compile()
    return nc


_CACHE = {}


def _get_nc():
    if "nc" not in _CACHE:
        _CACHE["nc"] = _build_nc()
    return _CACHE["nc"]


def make_in_maps(x, Wq, bq, Wk, bk, Wv, bv, Wo, bo):
    """Host-side sharding: per-core input dicts for cores 0..7."""
    x = np.asarray(x, np.float32)
    scale = np.float32(1.0 / np.sqrt(HD))
    Wq_s = np.asarray(Wq, np.float32) * scale
    bq_s = np.asarray(bq, np.float32) * scale
    Wk = np.asarray(Wk, np.float32)
    bk = np.asarray(bk, np.float32)
    Wv = np.asarray(Wv, np.float32)
    Wo = np.asarray(Wo, np.float32)

    def chunk_rows(w):  # [1024, M] -> [128, 8, M]
        return np.ascontiguousarray(
            w.reshape(NDC, P, w.shape[1]).transpose(1, 0, 2)).astype(NPDT)

    # x^T seq-major: [128 d-in-chunk, 4 seq-block, 8 d-chunk, 512 seq]
    xds = [np.ascontiguousarray(
               chunk_rows(x[b].T).reshape(P, NDC, NQB, QB).transpose(0, 2, 1, 3))
           for b in range(2)]
    in_maps = []
    for core in range(8):
        b, g = divmod(core, 4)
        cols = slice(g * GD, (g + 1) * GD)
        bias = np.zeros((P, 4), np.float32)
        bias[:, 0] = bq_s[g * GD:g * GD + P]
        bias[:, 1] = bq_s[g * GD + P:(g + 1) * GD]
        bias[:, 2] = bk[g * GD:g * GD + P]
        bias[:, 3] = bk[g * GD + P:(g + 1) * GD]
        in_maps.append({
            "xd": xds[b],
            "wq": chunk_rows(Wq_s[:, cols]),
            "wk": chunk_rows(Wk[:, cols]),
            "wv": chunk_rows(Wv[:, cols]),
            "wo": np.ascontiguousarray(
                Wo[cols, :].reshape(2, P, D).transpose(1, 0, 2)).astype(NPDT),
            "bias": bias,
        })
    return in_maps


def gather_output(results, Wv, bv, Wo, bo):
    """Sum per-core partial outputs and fold bv/bo exactly."""
    y = np.zeros((2, S, D), np.float32)
    for core in range(8):
        b = core // 4
        # yo [128 p, 4 qb, 8 m, 512 col] -> [qb*512+col, m*128+p] = [s, d]
        yo = np.asarray(results[core]["yo"], dtype=np.float32)
        y[b] += yo.transpose(1, 3, 2, 0).reshape(S, D)
    y += np.asarray(bo, np.float32) + np.asarray(bv, np.float32) @ np.asarray(Wo, np.float32)
    return y


def kernel(x, Wq, bq, Wk, bk, Wv, bv, Wo, bo):
    global LAST_RESULTS
    from concourse.bass_utils import run_bass_kernel_spmd
    in_maps = make_in_maps(x, Wq, bq, Wk, bk, Wv, bv, Wo, bo)
    res = run_bass_kernel_spmd(_get_nc(), in_maps, core_ids=list(range(8)),
                               trace=TRACE)
    LAST_RESULTS = res
    return gather_output(res.results, Wv, bv, Wo, bo)


# revision 30
# speedup vs baseline: 1.2242x; 1.0235x over previous
"""Trainium2 Bass kernel for a 16-head self-attention block.

Model (matches the nn.Module reference):
    q = x @ Wq + bq; k = x @ Wk + bk; v = x @ Wv + bv   (per-head split, Hd=64)
    attn = softmax(q k^T / sqrt(Hd)); out = (attn v) @ Wo + bo
Shapes: x [2, 2048, 1024], 16 heads, head dim 64.

Sharding (8 cores): core = (batch b in {0,1}) x (head-group g in {0..3});
each core owns 4 heads of one batch element. Inputs are sliced on the host;
each core returns a partial y^T = (attended_g @ Wo_g)^T which the host sums
over the 4 head-groups per batch.

Per-core design (all PE operands bf16; PSUM/normalization fp32):
  - Host passes x^T pre-chunked SEQ-MAJOR [128, 4 seq-blocks, 8 d-chunks,
    512] so the first Q/K projection slabs (and the whole qb0 stream) gate
    on just the first 1MB x block, not the whole 4MB tensor. Every
    projection consumer needs (all d-chunks x one seq block), so per-block
    DMAs unlock compute incrementally with 8KB-per-partition descriptors.
  - Input DMAs are ONE dma_start per tensor/block: the hardware sprays a
    [128, ...] transfer across all 16 DMA engines on its own, and each
    dma_start costs ~0.7us of issue time on its queue, so many small DMAs
    serialize on the issuing engine (the old 33-issue prologue spent ~25us
    just issuing). Weight DMAs issue from GpSimd, x blocks from Sync.
  - Scores are computed transposed, S^T[key, q] = K_h Q_h^T, so softmax's
    exp runs straight out of PSUM on the Scalar engine and A = P V consumes
    P^T with no transpose anywhere. Two heads of a pair share each score
    matmul slab via PE row groups (K=64 at row offsets 0/64).
  - softmax skips the max subtraction (mathematically identical; scores are
    O(5) here and ACT exp is <=2 ULP on [-10,10]).
  - P row sums ride the A = P V matmul via a ones column in V ([V|1] ->
    rows 0..63 attended + row 64 sums).
  - The exp (Scalar/ACT) stream is the pacing engine in steady state
    (~1.4us per key chunk, 128 chunks). To keep it stall-free the attended
    accumulators are RELEASED FAST: right after the AV stop-chunk, DVE
    casts av_ps[h][0:64] to SBUF (bf16) and copies the sums row to
    partition 0 (fp32); the PSUM banks free ~1.4us after the last AV
    instead of ~4us after the full normalization chain, so the next pair's
    AV (which reuses the same 2 banks) never back-pressures the exp stream
    through the pt WAR. pt pool is 6 deep for the same reason.
  - Normalization off PSUM: reciprocal_approx_fast on DVE, partition-
    broadcast on the (otherwise idle) GpSimd engine, one DVE multiply per
    head (bf16 attended x fp32 broadcast -> bf16 at_pair).
  - 1/sqrt(Hd) is folded into Wq (and bq) on the host; bv and bo are folded
    in exactly on the host: y += bo + bv @ Wo (softmax rows sum to 1).
  - Output projection of block qb is emitted a few chunks into block qb+1
    at low priority so its matmuls fill PE slack. For the LAST block the
    evictions alternate Scalar/Vector (both idle by then) to shorten the
    tail, and each y block goes out as one whole-tensor spray DMA.
  - A short dummy-matmul chain at t=0 ramps the PE clock while input DMAs
    stream.
"""

import numpy as np
import ml_dtypes

import concourse.bass as bass
import concourse.tile as tile
from concourse import bacc
from concourse import mybir

P = 128          # partitions
S = 2048         # sequence length
D = 1024         # model dim
H = 16           # total heads
HD = 64          # head dim
G = 4            # heads per core
GD = G * HD      # 256 head-group dims per core
NQB = 4          # query blocks (= seq blocks)
QB = S // NQB    # 512
NKC = S // P     # 16 key chunks
NDC = D // P     # 8 contraction chunks
F32 = mybir.dt.float32
BF16 = mybir.dt.bfloat16
DT = BF16        # PE operand dtype
NPDT = ml_dtypes.bfloat16
N_WARM = 12      # PE clock-ramp dummy matmuls (covers the ~13us DMA prologue)

TRACE = False
LAST_RESULTS = None


def _build_nc():
    nc = bacc.Bacc(trn_type="TRN2")
    xd = nc.dram_tensor("xd", [P, NQB, NDC, QB], DT, kind="ExternalInput")
    wq = nc.dram_tensor("wq", [P, NDC, GD], DT, kind="ExternalInput")
    wk = nc.dram_tensor("wk", [P, NDC, GD], DT, kind="ExternalInput")
    wv = nc.dram_tensor("wv", [P, NDC, GD], DT, kind="ExternalInput")
    wo = nc.dram_tensor("wo", [P, 2, D], DT, kind="ExternalInput")
    bias = nc.dram_tensor("bias", [P, 4], F32, kind="ExternalInput")
    yo = nc.dram_tensor("yo", [P, NQB, NDC, QB], DT, kind="ExternalOutput")

    Exp = mybir.ActivationFunctionType.Exp

    with tile.TileContext(nc) as tc, \
         tc.tile_pool(name="sb", bufs=1) as sb, \
         tc.tile_pool(name="pt", bufs=7) as ptp, \
         tc.tile_pool(name="attnp", bufs=5) as atp, \
         tc.tile_pool(name="avsbp", bufs=4) as avs, \
         tc.tile_pool(name="normp", bufs=4) as nrm, \
         tc.tile_pool(name="ysbp", bufs=2) as ysp, \
         tc.tile_pool(name="ps_s", bufs=2, space="PSUM") as ps_s, \
         tc.tile_pool(name="ps_av", bufs=2, space="PSUM") as ps_av, \
         tc.tile_pool(name="ps_y", bufs=2, space="PSUM") as ps_y:

        # ---- persistent SBUF tensors
        wq_sb = sb.tile([P, NDC, GD], DT, tag="wq")
        wk_sb = sb.tile([P, NDC, GD], DT, tag="wk")
        wv_sb = sb.tile([P, NDC, GD], DT, tag="wv")
        wo_sb = sb.tile([P, 2, D], DT, tag="wo")   # [pair-dims, pair, out-dim]
        bias_sb = sb.tile([P, 4], F32, tag="bias")
        scratch = sb.tile([P, 1], F32, tag="scratch")
        warm = sb.tile([P, QB], DT, tag="warm")
        x_sb = sb.tile([P, NQB, NDC, QB], DT, tag="x")
        kT = [sb.tile([P, S], DT, tag=f"k{p}", name=f"k{p}") for p in range(2)]
        qT = [sb.tile([P, S], DT, tag=f"q{p}", name=f"q{p}") for p in range(2)]
        # V with a ones column per head: [keys, chunk, head, 65] = [V | 1]
        v_sb = sb.tile([P, NKC, G, HD + 1], DT, tag="v")

        # ---- warm tiles + PE ramp (no input deps: runs during the DMA
        # prologue so the PE clock is at speed when projections start)
        nc.vector.memset(warm, 0.0)
        nc.vector.memset(v_sb[:, :, :, HD:HD + 1], 1.0)
        # warm the exp table set early so the ~2.7us load overlaps the DMAs
        nc.scalar.activation(out=scratch, in_=warm[:, 0:1], func=Exp)
        with tc.high_priority(offset=-1000000):
            for i in range(N_WARM):
                wps = ps_y.tile([P, QB], F32, tag="y", name="warm_ps")
                nc.tensor.matmul(wps[:], lhsT=warm[:, 0:P], rhs=warm[:],
                                 start=True, stop=True)

        # ---- input DMAs. One spray-DMA per tensor / x block (the DMA
        # hardware sprays each [128, ...] transfer across all 16 engines);
        # per-queue FIFO makes issue order the arrival order, so both queues
        # are laid out in consumption order: weights on GpSimd (wq before
        # wk — the first projection slab is Q), x blocks on Sync. The two
        # queues share HBM bandwidth ~evenly, landing wq ~8us in and x
        # block b at ~10+5b us — each just ahead of its first consumer.
        # (Keep the two-queue split: routing everything through one queue
        # measurably slowed every ACTIVATE by ~220ns and serialized the
        # Sync engine. And use the Scalar HW queue for weights, NOT the
        # GpSimd software queue — the latter moves only ~55 GB/s.)
        # wq/wk must beat x block 0 (they gate the very first projections)
        # so they lead the fast Sync queue; wv/wo ride the slower Scalar
        # queue concurrently and still land well before their consumers.
        nc.sync.dma_start(out=wq_sb, in_=wq[:, :, :])
        nc.sync.dma_start(out=x_sb[:, 0], in_=xd[:, 0])
        nc.sync.dma_start(out=wk_sb, in_=wk[:, :, :])
        nc.sync.dma_start(out=bias_sb, in_=bias[:, :])
        nc.scalar.dma_start(out=wv_sb, in_=wv[:, :, :])
        nc.scalar.dma_start(out=wo_sb, in_=wo[:, :, :])
        for b in range(1, NQB):
            nc.sync.dma_start(out=x_sb[:, b], in_=xd[:, b])

        # Pre-observe each weight DMA on the PE with a 1x1 dummy matmul, so
        # real matmuls never need two DMA-queue waits at once (walrus can't
        # encode >1 sync wait on an LDWEIGHTS; dropping these measurably
        # slowed every ACTIVATE by ~220ns). wq/wk are observed here — they
        # land first on the fast queue — while the slow-queue wv/wo
        # observations are emitted right before their first consumers so
        # they never stall the projection stream.
        wtouch_ps = ps_y.tile([1, 4], F32, tag="y", name="wtouch")

        def wtouch(w, i):
            nc.tensor.matmul(wtouch_ps[:, i:i + 1],
                             lhsT=w[0:1, 0, 0:1],
                             rhs=w[0:1, 0, 0:1],
                             start=True, stop=True)

        wtouch(wq_sb, 0)
        wtouch(wk_sb, 1)

        # ---- projection emitters
        def emit_qk_group(w_sb, dst, bcol0, p, blk):
            # one [128, 512] output slab of K^T or Q^T; dst[p] [128, 2048]
            # rows 64*h2 hold head (2p+h2)'s 64 dims, columns are sequence.
            # Depends on x seq-block blk only. Allocated from the ps_y pool
            # so the score pool is never blocked behind projection
            # evictions.
            n0 = blk * QB
            ps = ps_y.tile([P, QB], F32, tag="y", name="qk_ps")
            for d in range(NDC):
                nc.tensor.matmul(
                    ps[:],
                    lhsT=w_sb[:, d, p * P:(p + 1) * P],
                    rhs=x_sb[:, blk, d, :],
                    start=(d == 0), stop=(d == NDC - 1))
            # evict with per-partition bias add on the DVE (keeps the Scalar
            # engine free to run the exp stream from its very first chunk)
            with nc.allow_low_precision(reason="bf16 projection"):
                nc.vector.tensor_scalar_add(
                    out=dst[p][:, n0:n0 + QB],
                    in0=ps[:],
                    scalar1=bias_sb[:, bcol0 + p:bcol0 + p + 1])

        def emit_v_chunk(c):
            blk, c0 = c // 4, (c % 4) * P
            ps = ps_y.tile([P, GD], F32, tag="y", name="v_ps")
            for d in range(NDC):
                nc.tensor.matmul(
                    ps[:],
                    lhsT=x_sb[:, blk, d, c0:c0 + P],
                    rhs=wv_sb[:, d, :],
                    start=(d == 0), stop=(d == NDC - 1))
            nc.vector.tensor_copy(
                out=v_sb[:, c, :, 0:HD],
                in_=ps[:].rearrange("p (h d) -> p h d", h=G))

        # Engines execute their static streams IN ORDER, so every
        # projection group must be emitted at the point its x block lands —
        # never earlier (it would block the stream behind its DMA wait) and
        # never later than its first consumer. Only pair-0's slab-0 K and Q
        # groups and V chunk 0 precede the attention loop: the first exp —
        # which starts the Scalar stream that paces the whole kernel —
        # fires as soon as x block 0 is in. V chunks 1..15 and the later K
        # slabs interleave into the qb0 chunk loops right where needed.
        emit_qk_group(wq_sb, qT, 0, 0, 0)
        emit_qk_group(wk_sb, kT, 2, 0, 0)
        wtouch(wv_sb, 2)
        emit_v_chunk(0)

        # ---- attention + output projection: per query block, head pairs
        # processed sequentially (pass p covers heads 2p, 2p+1). The output
        # projection of block qb is emitted a few chunks into block qb+1 so
        # its matmuls fill PE slack instead of stalling the exp stream.
        pending_outproj = None
        for qb in range(NQB):
            q0 = qb * QB
            attn = []
            for p in range(2):
                av_ps = [ps_av.tile([P, QB], F32, tag="av", name="av_ps")
                         for _ in range(2)]
                for c in range(NKC):
                    if qb == 0 and p == 0:
                        # ALL qb0 projection work lives in pair 0's loop
                        # (which is PE-oversubscribed anyway): pair 1's K/Q
                        # slabs here keep pair 1's stream pure attention so
                        # the p0->p1 handoff costs the exp stream nothing.
                        if c in (4, 8, 12):
                            emit_qk_group(wk_sb, kT, 2, 0, c // 4)
                        if c in (5, 9, 13):
                            emit_qk_group(wk_sb, kT, 2, 1, c // 4)
                        if c >= 1:
                            emit_v_chunk(c)  # V just ahead of its first AV
                        if c == 1:
                            emit_qk_group(wk_sb, kT, 2, 1, 0)
                        if c == 3:
                            emit_qk_group(wq_sb, qT, 0, 1, 0)
                        if c == 2:
                            # qb1 queries; x block 1 nearly in — slack filler
                            with tc.high_priority(offset=-1000000):
                                emit_qk_group(wq_sb, qT, 0, 0, 1)
                        if c == 6:
                            with tc.high_priority(offset=-1000000):
                                emit_qk_group(wq_sb, qT, 0, 1, 1)
                        if c == 8:
                            wtouch(wo_sb, 3)
                    if pending_outproj is not None and p == 0 and c == 3:
                        pending_outproj()
                        pending_outproj = None
                    c0 = c * P
                    s_ps = ps_s.tile([P, 2, QB], F32, tag="s")
                    for h2 in range(2):
                        base = HD * h2
                        nc.tensor.matmul(
                            s_ps[:, h2],
                            lhsT=kT[p][base:base + HD, c0:c0 + P],
                            rhs=qT[p][base:base + HD, q0:q0 + QB],
                            start=True, stop=True,
                            tile_position=(base, 0))
                    pt = ptp.tile([P, 2, QB], DT, tag="pt")
                    nc.scalar.activation(out=pt[:], in_=s_ps[:], func=Exp)
                    for h2 in range(2):
                        nc.tensor.matmul(
                            av_ps[h2][0:HD + 1, :],
                            lhsT=v_sb[:, c, 2 * p + h2, :],
                            rhs=pt[:, h2],
                            start=(c == 0), stop=(c == NKC - 1))

                # Normalization. The av tile packs both heads side by side
                # (banks h=0/1), so one DVE op covers both heads for the
                # attended-rows cast to SBUF bf16 (FIRST, so the 2 av banks
                # release ~2.7us after the last AV matmul — the next pair's
                # AV start rides on that while the 7-deep pt pool keeps the
                # exp stream ahead) and for the sums-row copy to partition 0
                # (fp32; a plain DVE copy can partition-base shift, the
                # custom recip op cannot). Reciprocal on DVE, partition
                # broadcast on the otherwise-idle GpSimd, one DVE multiply
                # per head. For the LAST pair there is no next-pair release
                # pressure, so the cast is skipped and the multiplies read
                # the attended rows straight from PSUM.
                last = qb == NQB - 1 and p == 1
                if last:
                    # keep the PE clock hot through the final normalization
                    # chain (~5us of otherwise-idle PE would re-throttle the
                    # HAM and run the last output projection at half clock).
                    # Reading the last pt tile pins these after the final
                    # exp — dependency-free fillers get list-scheduled into
                    # earlier slack and miss this window entirely.
                    for i in range(10):
                        wps = ps_y.tile([P, QB], F32, tag="y",
                                        name="tail_warm_ps")
                        nc.tensor.matmul(wps[:], lhsT=warm[:, 0:P],
                                         rhs=pt[:, 0], start=True, stop=True)
                av_sb = [avs.tile([HD, QB], DT, tag=f"avsb{h}",
                                  name=f"avsb{h}") for h in range(2)]
                rr = [nrm.tile([1, QB], F32, tag=f"rr{h}", name=f"rr{h}")
                      for h in range(2)]
                rc = [nrm.tile([1, QB], F32, tag=f"rc{h}", name=f"rc{h}")
                      for h in range(2)]
                bc = [nrm.tile([HD, QB], F32, tag=f"bc{h}", name=f"bc{h}")
                      for h in range(2)]
                at_pair = atp.tile([P, QB], DT, tag="attn")
                with nc.allow_low_precision(reason="softmax denom approx"):
                    for h in range(2):
                        if not last:
                            nc.vector.tensor_copy(out=av_sb[h][:],
                                                  in_=av_ps[h][0:HD, :])
                        nc.vector.tensor_copy(out=rr[h][:],
                                              in_=av_ps[h][HD:HD + 1, :])
                        nc.vector.reciprocal_approx_fast(out=rc[h][:],
                                                         in_=rr[h][:])
                        nc.gpsimd.partition_broadcast(bc[h][:, :], rc[h][:, :])
                    for h in range(2):
                        # partition-base shift 0 -> 64 on the DVE packs the
                        # odd head into the pair tile with no relocation DMA
                        nc.vector.tensor_tensor(
                            out=at_pair[h * HD:(h + 1) * HD, :],
                            in0=(av_ps[h][0:HD, :] if last
                                 else av_sb[h][:]),
                            in1=bc[h][:, :],
                            op=mybir.AluOpType.mult)
                attn.append(at_pair)
                # qb2/qb3 queries, deprioritized so they only fill PE slack
                # in the later (projection-free, ACT-paced) query blocks
                if qb in (1, 2):
                    with tc.high_priority(offset=-1000000):
                        emit_qk_group(wq_sb, qT, 0, p, qb + 1)

            def emit_outproj(attn=attn, qb=qb):
                # y^T[m-chunk, qb] = sum_p Wo_p^T @ attn_pair_p.
                # Deprioritized: these matmuls fill PE slack so they never
                # delay the score matmuls that feed the exp stream. For the
                # last block each eviction is split across Scalar+Vector
                # (both idle once the exp stream ends) so the yp PSUM pair
                # recycles ~2x faster, and the y DMA goes out in two halves
                # to overlap the final evictions.
                last = qb == NQB - 1
                ctx2 = tc.high_priority(offset=-1000000)
                ctx2.__enter__()
                ysb = ysp.tile([P, NDC, QB], DT, tag="ysb")
                for m in range(NDC):
                    if last and m % 2 == 1:
                        # the score pool is free once the exp stream ends;
                        # alternating yp between the two pools doubles the
                        # eviction double-buffering so the final output
                        # projection is matmul-paced, not eviction-paced
                        yp = ps_s.tile([P, 2, QB], F32, tag="s",
                                       name="yp_s")[:, 0, :]
                    else:
                        yp = ps_y.tile([P, QB], F32, tag="y", name="yp")
                    for h in range(2):
                        nc.tensor.matmul(
                            yp[:],
                            lhsT=wo_sb[:, h, m * P:(m + 1) * P],
                            rhs=attn[h][:],
                            start=(h == 0), stop=(h == 1))
                    with nc.allow_low_precision(reason="bf16 partial out"):
                        if last:
                            nc.scalar.copy(out=ysb[:, m, 0:QB // 2],
                                           in_=yp[:, 0:QB // 2])
                            nc.vector.tensor_copy(out=ysb[:, m, QB // 2:],
                                                  in_=yp[:, QB // 2:])
                        else:
                            nc.vector.tensor_copy(out=ysb[:, m, :], in_=yp[:])
                    if last and m == NDC // 2 - 1:
                        nc.sync.dma_start(out=yo[:, qb, 0:NDC // 2, :],
                                          in_=ysb[:, 0:NDC // 2, :])
                # whole-tensor spray DMAs (8KB per-partition descriptors)
                if last:
                    nc.sync.dma_start(out=yo[:, qb, NDC // 2:, :],
                                      in_=ysb[:, NDC // 2:, :])
                else:
                    nc.sync.dma_start(out=yo[:, qb, :, :], in_=ysb[:, :, :])
                ctx2.__exit__(None, None, None)

            pending_outproj = emit_outproj

        if pending_outproj is not None:
            pending_outproj()

    nc.compile()
    return nc


_CACHE = {}


def _get_nc():
    if "nc" not in _CACHE:
        _CACHE["nc"] = _build_nc()
    return _CACHE["nc"]


def make_in_maps(x, Wq, bq, Wk, bk, Wv, bv, Wo, bo):
    """Host-side sharding: per-core input dicts for cores 0..7."""
    x = np.asarray(x, np.float32)
    scale = np.float32(1.0 / np.sqrt(HD))
    Wq_s = np.asarray(Wq, np.float32) * scale
    bq_s = np.asarray(bq, np.float32) * scale
    Wk = np.asarray(Wk, np.float32)
    bk = np.asarray(bk, np.float32)
    Wv = np.asarray(Wv, np.float32)
    Wo = np.asarray(Wo, np.float32)

    def chunk_rows(w):  # [1024, M] -> [128, 8, M]
        return np.ascontiguousarray(
            w.reshape(NDC, P, w.shape[1]).transpose(1, 0, 2)).astype(NPDT)

    # x^T seq-major: [128 d-in-chunk, 4 seq-block, 8 d-chunk, 512 seq]
    xds = [np.ascontiguousarray(
               chunk_rows(x[b].T).reshape(P, NDC, NQB, QB).transpose(0, 2, 1, 3))
           for b in range(2)]
    in_maps = []
    for core in range(8):
        b, g = divmod(core, 4)
        cols = slice(g * GD, (g + 1) * GD)
        bias = np.zeros((P, 4), np.float32)
        bias[:, 0] = bq_s[g * GD:g * GD + P]
        bias[:, 1] = bq_s[g * GD + P:(g + 1) * GD]
        bias[:, 2] = bk[g * GD:g * GD + P]
        bias[:, 3] = bk[g * GD + P:(g + 1) * GD]
        in_maps.append({
            "xd": xds[b],
            "wq": chunk_rows(Wq_s[:, cols]),
            "wk": chunk_rows(Wk[:, cols]),
            "wv": chunk_rows(Wv[:, cols]),
            "wo": np.ascontiguousarray(
                Wo[cols, :].reshape(2, P, D).transpose(1, 0, 2)).astype(NPDT),
            "bias": bias,
        })
    return in_maps


def gather_output(results, Wv, bv, Wo, bo):
    """Sum per-core partial outputs and fold bv/bo exactly."""
    y = np.zeros((2, S, D), np.float32)
    for core in range(8):
        b = core // 4
        # yo [128 p, 4 qb, 8 m, 512 col] -> [qb*512+col, m*128+p] = [s, d]
        yo = np.asarray(results[core]["yo"], dtype=np.float32)
        y[b] += yo.transpose(1, 3, 2, 0).reshape(S, D)
    y += np.asarray(bo, np.float32) + np.asarray(bv, np.float32) @ np.asarray(Wo, np.float32)
    return y


def kernel(x, Wq, bq, Wk, bk, Wv, bv, Wo, bo):
    global LAST_RESULTS
    from concourse.bass_utils import run_bass_kernel_spmd
    in_maps = make_in_maps(x, Wq, bq, Wk, bk, Wv, bv, Wo, bo)
    res = run_bass_kernel_spmd(_get_nc(), in_maps, core_ids=list(range(8)),
                               trace=TRACE)
    LAST_RESULTS = res
    return gather_output(res.results, Wv, bv, Wo, bo)
